# revision 15
# baseline (speedup 1.0000x reference)
"""Multi-Head Latent Attention kernel for 8 Trainium2 NeuronCores.

Sharding: batch (4) x head-halves (2) -> 8 cores. Each core handles one batch
element and 8 of 16 heads: RMSNorm (fused into a scaled PE transpose), q/kv/k/v
projections, RoPE, full attention with a fused PV+rowsum matmul (stationary
[V_h | ones], M=65), and its slice of the output projection. Host folds norm_w
into wq/kv_down, adds the residual, and sums the two head-half partials.

All matmuls run in float32r (fp32 storage, full PE rate, ~1e-4 rounding).
"""
import math

import numpy as np

import concourse.bacc as bacc
import concourse.tile as tile
from concourse import mybir
from concourse.bass import ts
from concourse.bass_utils import run_bass_kernel_spmd

B, S, D = 4, 2048, 1024
H, HD, L = 16, 64, 512
ROPE_BASE = 10000.0
EPS = 1e-6
P = 128
SC = 512               # s-chunk width
NCH = S // SC          # 4 chunks
NSB = SC // P          # 4 s-blocks per chunk
NT = S // P            # 16 t-blocks
DK = D // P            # 8 k-tiles over D
LK = L // P            # 4 k-tiles over L
NPAIR = 4              # head pairs per core
SCALE = 1.0 / math.sqrt(HD)
F32 = mybir.dt.float32
F32R = mybir.dt.float32r

_CACHE = {}


def _build_nc(stop_after=None):
    nc = bacc.Bacc("TRN2", target_bir_lowering=False, debug=False)
    x_d = nc.dram_tensor("x", (S, D), F32R, kind="ExternalInput")
    wq_d = nc.dram_tensor("wq", (D, L), F32R, kind="ExternalInput")
    kvd_d = nc.dram_tensor("kvd", (D, L), F32R, kind="ExternalInput")
    wk_d = nc.dram_tensor("wk", (L, L), F32R, kind="ExternalInput")
    wv_d = nc.dram_tensor("wv", (L, L), F32R, kind="ExternalInput")
    wo_d = nc.dram_tensor("wo", (L, D), F32R, kind="ExternalInput")
    cos_d = nc.dram_tensor("cos_t", (P, S), F32R, kind="ExternalInput")
    sin_d = nc.dram_tensor("sin_t", (P, S), F32R, kind="ExternalInput")
    ident_d = nc.dram_tensor("ident", (P, P), F32R, kind="ExternalInput")
    p32_d = nc.dram_tensor("p32", (P, P), F32R, kind="ExternalInput")
    bc2_d = nc.dram_tensor("bc2", (64, P), F32R, kind="ExternalInput")
    out_d = nc.dram_tensor("out", (S, D), F32, kind="ExternalOutput")

    with tile.TileContext(nc) as tc:
        with tc.tile_pool(name="const", bufs=1) as cpool, \
             tc.tile_pool(name="persist", bufs=1) as ppool:
            ident_sb = cpool.tile([P, P], F32R)
            nc.sync.dma_start(ident_sb[:], ident_d[:])
            p32_sb = cpool.tile([P, P], F32R)
            nc.sync.dma_start(p32_sb[:], p32_d[:])
            bc2_sb = cpool.tile([64, P], F32R)
            nc.sync.dma_start(bc2_sb[:], bc2_d[:])

            # persistent: qT/kT pair tiles (128 d, S), v tiles (128 t, 8 heads, 65)
            qT = kT = v_sb = None
            if stop_after not in ("tr", "kv"):
                qT = [ppool.tile([P, S], F32R, name=f"qT{p}") for p in range(NPAIR)]
            if stop_after not in ("tr", "kv", "q"):
                kT = [ppool.tile([P, S], F32R, name=f"kT{p}") for p in range(NPAIR)]
                v_sb = [ppool.tile([P, 2 * NPAIR, HD + 1], F32R, name=f"v{t}") for t in range(NT)]
            rr_sb = ppool.tile([64, SC], F32R)
            nc.vector.memset(rr_sb[:].bitcast(F32), 0.0)

            # ---------------- Phase 1: norm, transpose, projections, rope ----
            with tc.tile_pool(name="p1sb", bufs=1) as p1, \
                 tc.tile_pool(name="xin", bufs=5) as xpool, \
                 tc.tile_pool(name="xsq", bufs=1) as xsqpool, \
                 tc.tile_pool(name="diagp", bufs=2) as diagpool, \
                 tc.tile_pool(name="wstream", bufs=2) as wpool, \
                 tc.tile_pool(name="xnt", bufs=1) as xntp, \
                 tc.tile_pool(name="kvt", bufs=1) as kvtp, \
                 tc.tile_pool(name="ropetmp", bufs=2) as rtp, \
                 tc.tile_pool(name="trps", bufs=2, space="PSUM") as trps, \
                 tc.tile_pool(name="projps", bufs=1, space="PSUM") as projps, \
                 tc.tile_pool(name="swps", bufs=2, space="PSUM") as swps:
                cos_sb = p1.tile([P, S], F32R)
                nc.sync.dma_start(cos_sb[:], cos_d[:])
                sin_sb = p1.tile([P, S], F32R)
                nc.sync.dma_start(sin_sb[:], sin_d[:])
                wk_sb = [p1.tile([P, L], F32R, name=f"wk{l}") for l in range(LK)]
                for l in range(LK):
                    nc.sync.dma_start(wk_sb[l][:], wk_d[ts(l, P), :])
                wv_sb = [p1.tile([P, L], F32R, name=f"wv{l}") for l in range(LK)]
                for l in range(LK):
                    nc.sync.dma_start(wv_sb[l][:], wv_d[ts(l, P), :])
                ssq = p1.tile([P, NSB], F32)
                std = p1.tile([P, NSB], F32)
                rs = p1.tile([P, NSB], F32)
                eps_sb = p1.tile([P, 1], F32)
                nc.vector.memset(eps_sb[:], EPS)

                def rope(src_ps, dst_ap, ch):
                    t1 = rtp.tile([P, SC], F32, tag="ropetc")
                    nc.vector.tensor_tensor(t1[:], src_ps[:], cos_sb[:, ts(ch, SC)],
                                            mybir.AluOpType.mult)
                    t2 = rtp.tile([P, SC], F32R, tag="ropets")
                    nc.vector.tensor_tensor(t2[:], src_ps[:], sin_sb[:, ts(ch, SC)],
                                            mybir.AluOpType.mult)
                    sw = swps.tile([P, SC], F32, tag="swps")
                    nc.tensor.matmul(sw[:], p32_sb[:], t2[:], start=True, stop=True)
                    nc.vector.tensor_tensor(dst_ap, sw[:], t1[:], mybir.AluOpType.add)

                for c in range(NCH):
                    xts = []
                    for i in range(NSB):
                        xt = xpool.tile([P, D], F32R, tag="x")
                        nc.sync.dma_start(xt[:], x_d[ts(NSB * c + i, P), :])
                        scratch = xsqpool.tile([P, D], F32, tag="xsq")
                        nc.scalar.activation(scratch[:], xt[:].bitcast(F32),
                                             mybir.ActivationFunctionType.Square,
                                             accum_out=ssq[:, i:i + 1])
                        xts.append(xt)
                    nc.scalar.activation(std[:], ssq[:], mybir.ActivationFunctionType.Sqrt,
                                         bias=eps_sb[:], scale=1.0 / D)
                    nc.vector.reciprocal(rs[:], std[:])
                    # transpose with fused row scale: xnT[j][:, i] = x_i[:, j].T @ diag(rs_i)
                    xnt = [xntp.tile([P, SC], F32R, tag=f"xnt{j}", name=f"xnt{j}") for j in range(DK)]
                    for i in range(NSB):
                        diag = diagpool.tile([P, P], F32R, tag="diag")
                        nc.vector.tensor_scalar_mul(diag[:], ident_sb[:], rs[:, i:i + 1])
                        for j in range(DK):
                            tp = trps.tile([P, P], F32, tag="trps")
                            nc.tensor.matmul(tp[:], xts[i][:, ts(j, P)], diag[:],
                                             start=True, stop=True)
                            nc.scalar.copy(xnt[j][:, ts(i, P)], tp[:])
                    if stop_after == "tr":
                        for j in range(DK):
                            nc.sync.dma_start(out_d[ts(NSB * c + (j % NSB), P), ts(j // NSB, SC)], xnt[j][:].bitcast(F32))
                        continue
                    # kvT chunk: contraction over D, j-outer with 4 m psums
                    kv_ps = [projps.tile([P, SC], F32, tag=f"pp{m}", name=f"kvps{m}") for m in range(LK)]
                    for j in range(DK):
                        kvw = wpool.tile([P, L], F32R, tag="kvd")
                        nc.sync.dma_start(kvw[:], kvd_d[ts(j, P), :])
                        for m in range(LK):
                            nc.tensor.matmul(kv_ps[m][:], kvw[:, ts(m, P)], xnt[j][:],
                                             start=(j == 0), stop=(j == DK - 1))
                    kvt = [kvtp.tile([P, SC], F32R, tag=f"kvt{m}", name=f"kvt{m}") for m in range(LK)]
                    for m in range(LK):
                        nc.scalar.copy(kvt[m][:], kv_ps[m][:])
                    if stop_after == "kv":
                        for m in range(LK):
                            nc.sync.dma_start(out_d[ts(NSB * c + m, P), ts(0, SC)], kvt[m][:].bitcast(F32))
                        continue
                    # qT chunk: contraction over D
                    q_ps = [projps.tile([P, SC], F32, tag=f"pp{p}", name=f"qps{p}") for p in range(NPAIR)]
                    for j in range(DK):
                        qw = wpool.tile([P, L], F32R, tag="wq")
                        nc.sync.dma_start(qw[:], wq_d[ts(j, P), :])
                        for p in range(NPAIR):
                            nc.tensor.matmul(q_ps[p][:], qw[:, ts(p, P)], xnt[j][:],
                                             start=(j == 0), stop=(j == DK - 1))
                    for p in range(NPAIR):
                        rope(q_ps[p], qT[p][:, ts(c, SC)], c)
                    if stop_after == "q":
                        continue
                    # kT chunk: contraction over L
                    k_ps = [projps.tile([P, SC], F32, tag=f"pp{p}", name=f"kps{p}") for p in range(NPAIR)]
                    for l in range(LK):
                        for p in range(NPAIR):
                            nc.tensor.matmul(k_ps[p][:], wk_sb[l][:, ts(p, P)], kvt[l][:],
                                             start=(l == 0), stop=(l == LK - 1))
                    for p in range(NPAIR):
                        rope(k_ps[p], kT[p][:, ts(c, SC)], c)
                    # v for the 4 t-blocks of this chunk: contraction over L
                    v_ps = [projps.tile([P, SC], F32, tag=f"pp{i}", name=f"vps{i}") for i in range(NSB)]
                    for l in range(LK):
                        for i in range(NSB):
                            nc.tensor.matmul(v_ps[i][:], kvt[l][:, ts(i, P)], wv_sb[l][:],
                                             start=(l == 0), stop=(l == LK - 1))
                    for i in range(NSB):
                        t = NSB * c + i
                        nc.vector.memset(v_sb[t][:].bitcast(F32), 1.0)
                        nc.vector.tensor_copy(
                            v_sb[t][:, :, 0:HD],
                            v_ps[i][:].rearrange("p (h e) -> p h e", h=2 * NPAIR))

            # ---------------- Phase 2: attention + output projection ---------
            if stop_after == "p1":
                for p in range(NPAIR):
                    nc.sync.dma_start(out_d[ts(p, P), :], qT[p][:, 0:D].bitcast(F32))
                    nc.sync.dma_start(out_d[ts(NPAIR + p, P), :], kT[p][:, 0:D].bitcast(F32))
            elif stop_after is None:
                _phase2(nc, tc, qT, kT, v_sb, rr_sb, bc2_sb, wo_d, out_d, stop_after)
    nc.compile()
    return nc


def _phase2(nc, tc, qT, kT, v_sb, rr_sb, bc2_sb, wo_d, out_d, stop_after):
            with tc.tile_pool(name="p2sb", bufs=1) as p2, \
                 tc.tile_pool(name="ptpool", bufs=2) as ptp, \
                 tc.tile_pool(name="atn", bufs=8) as atp, \
                 tc.tile_pool(name="outp", bufs=4) as outp, \
                 tc.tile_pool(name="scps", bufs=1, space="PSUM") as scps, \
                 tc.tile_pool(name="pvps", bufs=1, space="PSUM") as pvps, \
                 tc.tile_pool(name="bcps", bufs=1, space="PSUM") as bcps, \
                 tc.tile_pool(name="wops", bufs=1, space="PSUM") as wops:
                wo_sb = [p2.tile([P, D], F32R, name=f"wo{p}") for p in range(NPAIR)]
                for p in range(NPAIR):
                    nc.sync.dma_start(wo_sb[p][:], wo_d[ts(p, P), :])
                for c in range(NCH):
                    atn_c = []
                    for p in range(NPAIR):
                        pvA = pvps.tile([P, SC], F32, tag="pvA", name="pvA")
                        pvB = pvps.tile([P, SC], F32, tag="pvB", name="pvB")
                        for tg in range(NT // 2):
                            scA = scps.tile([P, 2, SC], F32, tag="scA", name="scA")
                            scB = scps.tile([P, 2, SC], F32, tag="scB", name="scB")
                            for u in range(2):
                                t = 2 * tg + u
                                nc.tensor.matmul(scA[:, u], kT[p][0:64, ts(t, P)],
                                                 qT[p][0:64, ts(c, SC)], start=True, stop=True)
                                nc.tensor.matmul(scB[:, u], kT[p][64:128, ts(t, P)],
                                                 qT[p][64:128, ts(c, SC)], start=True, stop=True)
                            ptA = ptp.tile([P, 2, SC], F32R, tag="ptA")
                            nc.scalar.activation(ptA[:], scA[:],
                                                 mybir.ActivationFunctionType.Exp, scale=SCALE)
                            ptB = ptp.tile([P, 2, SC], F32R, tag="ptB")
                            nc.scalar.activation(ptB[:], scB[:],
                                                 mybir.ActivationFunctionType.Exp, scale=SCALE)
                            for u in range(2):
                                t = 2 * tg + u
                                nc.tensor.matmul(pvA[0:HD + 1, :], v_sb[t][:, 2 * p, :],
                                                 ptA[:, u], start=(t == 0), stop=(t == NT - 1))
                                nc.tensor.matmul(pvB[0:HD + 1, :], v_sb[t][:, 2 * p + 1, :],
                                                 ptB[:, u], start=(t == 0), stop=(t == NT - 1))
                        with nc.allow_low_precision(reason="softmax denom reciprocal to f32r"):
                            nc.vector.reciprocal(rr_sb[0:1, :], pvA[HD:HD + 1, :])
                            nc.vector.reciprocal(rr_sb[32:33, :], pvB[HD:HD + 1, :])
                        bc_ps = bcps.tile([P, SC], F32, tag="bc")
                        nc.tensor.matmul(bc_ps[:], bc2_sb[:], rr_sb[:], start=True, stop=True)
                        bc_sb = outp.tile([P, SC], F32, tag="bcsb")
                        nc.vector.tensor_copy(bc_sb[:], bc_ps[:])
                        atn = atp.tile([P, SC], F32R, tag="atn")
                        nc.vector.tensor_tensor(atn[0:64, :], pvA[0:64, :], bc_sb[0:64, :],
                                                mybir.AluOpType.mult)
                        nc.vector.tensor_tensor(atn[64:128, :], pvB[0:64, :], bc_sb[64:128, :],
                                                mybir.AluOpType.mult)
                        atn_c.append(atn)
                    for sb in range(NSB):
                        for n in range(2):
                            wo_ps = wops.tile([P, SC], F32, tag="wops")
                            for p in range(NPAIR):
                                nc.tensor.matmul(wo_ps[:], atn_c[p][:, ts(sb, P)],
                                                 wo_sb[p][:, ts(n, SC)],
                                                 start=(p == 0), stop=(p == NPAIR - 1))
                            ot = outp.tile([P, SC], F32, tag="ot")
                            nc.vector.tensor_copy(ot[:], wo_ps[:])
                            nc.sync.dma_start(out_d[ts(NSB * c + sb, P), ts(n, SC)], ot[:])


def _host_prep(x, norm_w, wq, kv_down, wk, wv, wo):
    """Fold norm_w into the D-side weights; build rope tables and constants."""
    wq_n = (norm_w[:, None] * wq).astype(np.float32)
    kvd_n = (norm_w[:, None] * kv_down).astype(np.float32)
    inv_freq = (1.0 / (ROPE_BASE ** (np.arange(0, HD, 2, dtype=np.float64) / HD)))
    ang = np.arange(S, dtype=np.float64)[:, None] * inv_freq[None, :]   # (S, 32)
    cos32 = np.cos(ang).astype(np.float32)                              # (S, 32)
    sin32 = np.sin(ang).astype(np.float32)
    cos_t = np.empty((P, S), dtype=np.float32)
    sin_t = np.empty((P, S), dtype=np.float32)
    for r in range(P):
        f = r % 32
        cos_t[r] = cos32[:, f]
        sgn = 1.0 if (r % 64) < 32 else -1.0
        sin_t[r] = sgn * sin32[:, f]
    ident = np.eye(P, dtype=np.float32)
    p32 = np.zeros((P, P), dtype=np.float32)
    for m in range(P):
        sw = m + 32 if (m % 64) < 32 else m - 32
        p32[sw, m] = 1.0
    bc2 = np.zeros((64, P), dtype=np.float32)
    bc2[0, 0:64] = 1.0
    bc2[32, 64:128] = 1.0
    return wq_n, kvd_n, cos_t, sin_t, ident, p32, bc2


def kernel(x, norm_w, wq, kv_down, wk, wv, wo, _bench=None, _stop_after=None):
    x = np.asarray(x, dtype=np.float32)
    norm_w = np.asarray(norm_w, dtype=np.float32)
    wq = np.asarray(wq, dtype=np.float32)
    kv_down = np.asarray(kv_down, dtype=np.float32)
    wk = np.asarray(wk, dtype=np.float32)
    wv = np.asarray(wv, dtype=np.float32)
    wo = np.asarray(wo, dtype=np.float32)

    wq_n, kvd_n, cos_t, sin_t, ident, p32, bc2 = _host_prep(
        x, norm_w, wq, kv_down, wk, wv, wo)

    key = ("nc", _stop_after)
    if key not in _CACHE:
        _CACHE[key] = _build_nc(_stop_after)
    nc = _CACHE[key]

    in_maps = []
    for core in range(8):
        b, hh = core // 2, core % 2
        sl = slice(hh * 512, (hh + 1) * 512)
        in_maps.append({
            "x": np.ascontiguousarray(x[b]),
            "wq": np.ascontiguousarray(wq_n[:, sl]),
            "kvd": kvd_n,
            "wk": np.ascontiguousarray(wk[:, sl]),
            "wv": np.ascontiguousarray(wv[:, sl]),
            "wo": np.ascontiguousarray(wo[sl, :]),
            "cos_t": cos_t,
            "sin_t": sin_t,
            "ident": ident,
            "p32": p32,
            "bc2": bc2,
        })
    kwargs = dict(_bench or {})
    res = run_bass_kernel_spmd(nc, in_maps, core_ids=list(range(8)), **kwargs)
    out = np.empty((B, S, D), dtype=np.float32)
    for b in range(B):
        out[b] = x[b] + res.results[2 * b]["out"] + res.results[2 * b + 1]["out"]
    if _bench is not None:
        _CACHE["last_result"] = res
    return out
